# revision 21
# baseline (speedup 1.0000x reference)
"""Haar DWT->single-branch-IDWT decoupling layer (DecouplingFlowLayer) on 8 trn2 cores.

Input  x: [32, 512, 100, 6] f32.
Output (X_l, X_h), each [32, 512, 100, 6]:
    for each even/odd seq pair (x_e, x_o) = (x[:, 2i], x[:, 2i+1]):
        avg = (x_e + x_o)/2 ; dif = (x_e - x_o)/2
        X_l[2i] = X_l[2i+1] = avg
        X_h[2i] = dif ; X_h[2i+1] = -dif

Sharding: pure data-parallel over batch, 4 batches/core, no communication.

Per-core layout trick: a core's shard [4, 512, 600] flattened C-order is
viewed as [512 rows x 2400]: DRAM rows [128t, 128t+128) are exactly batch
t, so each tile transfer is one fully-contiguous 1.23 MB DRAM range, and
SBUF partition p of tile t holds 4 consecutive seq rows of batch t (= 2
complete even/odd pairs, each pair 1200 contiguous floats).  The butterfly
happens in SBUF on 600-float column slices.

Raw bass (no TileContext): the Tile scheduler attaches >1 sync-wait to
single instructions, which this toolchain's walrus codegen rejects ("Too
many sync wait commands").  Manual semaphores keep every instruction at
<=1 wait and skip Tile's kernel-tail barrier.  Engine dataflow is a strict
DAG across engines (load -> ACT scale -> DVE butterfly -> store) with no
same-engine RAW hazards (TRN2 engines are deep pipelines; same-engine
program order does not order memory).  X_l's duplicated halves are written
by storing the same SBUF tile with two DMAs instead of an on-chip copy.
All 4 batch tiles are SBUF-resident, so there are no slot-reuse waits;
SP's single HWDGE FIFO ring alone saturates the ~358 GB/s per-core HBM
limit, which is the roofline for this memory-bound op.
"""

import contextlib

import numpy as np

import concourse.bass as bass
import concourse.mybir as mybir
from concourse import bass_utils

_B, _S, _N, _F = 32, 512, 100, 6
_NCORES = 8
_BPC = _B // _NCORES            # batches per core
_ROW = _N * _F                  # 600 floats per (b, s) row
_P = 128                        # SBUF partitions
_COLS = _BPC * _S * _ROW // _P  # 9600 floats per partition
_PAIR = 2 * _ROW                # 1200 floats: one even/odd seq pair
_NT = _BPC                     # tiles per pass: one tile = one batch
_W = _S * _ROW // _P            # 2400 floats per partition per tile
_K = _W // _PAIR                # even/odd pairs per tile per partition
_DR = _NT * _P                  # 512 DRAM rows per core view

_nc_cache = None


def _build_nc(reps=1):
    """One SPMD program, identical on all 8 cores.

    reps > 1 unrolls the whole pass N times (idempotent rewrites of the
    same outputs) for wall-clock slope benchmarking; the graded kernel
    uses reps=1.

    12 uniform 1.23 MB DMAs per pass (4 loads, 4 X_l stores, 4 X_h
    stores) keep the single HWDGE FIFO pipe saturated with minimal
    per-transfer overhead.  X_l's duplicated halves are materialized by
    a second DVE add into the odd slots (cheap, fully hidden) so every
    store is one contiguous [128, 2400] block.
    """
    f32 = mybir.dt.float32
    W, R, K = _W, _ROW, _K
    nc = bass.Bass("TRN2", debug=False, num_devices=_NCORES)
    # [512, 2400] row-major view of the per-core shard: rows [128t, 128t+128)
    # = batch t, so every tile's DRAM range is one fully-contiguous 1.23 MB
    # block (sequential addresses are the memory controller's best case;
    # the column-tile [128, 9600] view was 128 strided 9.6 KB chunks).
    x_d = nc.declare_dram_parameter("x", [_DR, _W], f32, isOutput=False)[:]
    l_d = nc.declare_dram_parameter("out_l", [_DR, _W], f32, isOutput=True)[:]
    h_d = nc.declare_dram_parameter("out_h", [_DR, _W], f32, isOutput=True)[:]

    with contextlib.ExitStack() as st:
        # One semaphore per column-tile load: a wait at that sem's current
        # maximum (16 per completed DMA) is exact.  A single shared sem
        # with intermediate thresholds would race: the 16 SDMA engines
        # each inc once per transfer, so a mixed count can reach 16*t
        # with transfer t-1 still in flight.
        s_in = [
            st.enter_context(nc.semaphore(f"s_in{t}")) for t in range(_NT)
        ]
        s_act = st.enter_context(nc.semaphore("s_act"))  # scale t done
        s_le = st.enter_context(nc.semaphore("s_le"))    # lt t ready
        s_he = st.enter_context(nc.semaphore("s_he"))    # ht t ready
        s_out = st.enter_context(nc.semaphore("s_out"))  # store completions
        xt = [
            st.enter_context(nc.sbuf_tensor(f"xt{t}", [_P, W], f32))
            for t in range(_NT)
        ]
        xs = [
            st.enter_context(nc.sbuf_tensor(f"xs{t}", [_P, W], f32))
            for t in range(_NT)
        ]
        lt = [
            st.enter_context(nc.sbuf_tensor(f"lt{t}", [_P, W], f32))
            for t in range(_NT)
        ]
        ht = [
            st.enter_context(nc.sbuf_tensor(f"ht{t}", [_P, W], f32))
            for t in range(_NT)
        ]
        warm = st.enter_context(nc.sbuf_tensor("warm", [_P, 1], f32))

        def pairs(handle, off):
            # [128, K, 600] view of a [128, W] SBUF tile: pair k's
            # even half at k*1200, odd half at k*1200+600.
            return bass.AP(
                handle[:].tensor, off, [[W, _P], [_PAIR, K], [1, R]]
            )

        n_stores = 2 * _NT  # per pass

        with nc.Block() as block:

            @block.sync
            def _(sync):
                # Single HWDGE FIFO ring carries all 12 transfers: the 16
                # SDMA engines are shared across rings anyway, so a second
                # ring adds no bandwidth; one ring gives a pure-read phase
                # followed by a pure-write phase (DRAM-optimal).
                for p in range(reps):
                    for t in range(_NT):
                        rows = slice(_P * t, _P * (t + 1))
                        sync.dma_start(
                            out=xt[t][:], in_=x_d[rows]
                        ).then_inc(s_in[t], 16)
                    for t in range(_NT):
                        rows = slice(_P * t, _P * (t + 1))
                        sync.wait_ge(s_le, _NT * p + t + 1)
                        sync.dma_start(
                            out=l_d[rows], in_=lt[t][:]
                        ).then_inc(s_out, 16)
                        sync.wait_ge(s_he, _NT * p + t + 1)
                        sync.dma_start(
                            out=h_d[rows], in_=ht[t][:]
                        ).then_inc(s_out, 16)
                # Final wait at the absolute max -> exact.
                sync.wait_ge(s_out, n_stores * 16 * reps)

            @block.scalar
            def _(scalar):
                # Warm the ACTIVATE(Copy) lookup table during the load
                # phase: the first ACT op pays a ~1.6 us cold-table load
                # on real HW (not modeled by the cost simulators), which
                # would otherwise sit on the first tile's critical chain.
                # Source is the framework's preamble-initialized const-0.
                scalar.mul(
                    warm[:], nc.const_aps.tensor(0.0, (_P, 1)), 0.5
                )
                for p in range(reps):
                    for t in range(_NT):
                        scalar.wait_ge(s_in[t], 16 * (p + 1))
                        scalar.mul(xs[t][:], xt[t][:], 0.5).then_inc(s_act)

            @block.vector
            def _(vector):
                for p in range(reps):
                    if p > 0:
                        # WAR guard: pass p-1's stores must have read lt/ht
                        # before pass p overwrites them (loads and stores
                        # are on different rings now, so ring FIFO order
                        # no longer covers this).  Exact: this is s_out's
                        # maximum until pass p's s_le/s_he fire.
                        vector.wait_ge(s_out, n_stores * 16 * p)
                    for t in range(_NT):
                        xs_e = pairs(xs[t], 0)
                        xs_o = pairs(xs[t], R)
                        vector.wait_ge(s_act, _NT * p + t + 1)
                        vector.tensor_add(pairs(lt[t], 0), xs_e, xs_o)
                        vector.tensor_add(
                            pairs(lt[t], R), xs_e, xs_o
                        ).then_inc(s_le)
                        vector.tensor_sub(pairs(ht[t], 0), xs_e, xs_o)
                        vector.tensor_sub(
                            pairs(ht[t], R), xs_o, xs_e
                        ).then_inc(s_he)

    return nc


def get_nc():
    global _nc_cache
    if _nc_cache is None:
        _nc_cache = _build_nc()
    return _nc_cache


def _shard(x):
    x = np.ascontiguousarray(np.asarray(x, dtype=np.float32))
    return [
        {"x": x[i * _BPC : (i + 1) * _BPC].reshape(_DR, _W)}
        for i in range(_NCORES)
    ]


def _unshard(results):
    xl = np.concatenate(
        [r["out_l"].reshape(_BPC, _S, _N, _F) for r in results], axis=0
    )
    xh = np.concatenate(
        [r["out_h"].reshape(_BPC, _S, _N, _F) for r in results], axis=0
    )
    return xl, xh


def kernel(x):
    in_maps = _shard(x)
    last_err = None
    for backoff in (0, 20, 45):
        # A transiently wedged exec unit (e.g. a prior process died
        # mid-custom-call) recovers after tens of seconds; retry with
        # backoff before giving up.
        if backoff:
            import time

            time.sleep(backoff)
        try:
            res = bass_utils.run_bass_kernel_spmd(
                get_nc(), in_maps, core_ids=list(range(_NCORES))
            )
            return _unshard(res.results)
        except Exception as e:
            last_err = e
    raise last_err
